# revision 1
# baseline (speedup 1.0000x reference)
"""DecodeDetections keypoint-decode kernel for Trainium2 (8 NeuronCores).

Computation (per box, 20 input channels -> 12 output channels):
  out[0:2]    = in[0:2]                                  (class scores)
  out[2+2k]   = (in[2+2k] * in[16] * in[14] + in[12]) * 512   k=0..4  (kp x)
  out[3+2k]   = (in[3+2k] * in[17] * in[15] + in[13]) * 512   k=0..4  (kp y)

Sharding: batch axis (32) split 4-per-core across 8 cores; inside a core the
(4*100000, 20) rows are tiled partition-major: tile t covers rows
[t*80000, (t+1)*80000), partition p holds rows [t*80000 + p*J, ... + J).
All DMAs are 128 x (J*C*4B) with long contiguous runs per partition.
"""

import sys

import numpy as np

if "/opt/trn_rl_repo" not in sys.path:
    sys.path.insert(0, "/opt/trn_rl_repo")

import concourse.bacc as bacc
import concourse.bass as bass
import concourse.mybir as mybir
from concourse.tile import TileContext

N_CORES = 8
B, N, C_IN = 32, 100000, 20
C_OUT = 12
B_PER_CORE = B // N_CORES
ROWS = B_PER_CORE * N  # 400000 rows per core
P = 128
SCALE = 512.0
F32 = mybir.dt.float32


# Per-tile boxes-per-partition. Small first tiles start compute early
# (short pipeline fill); small last tile shortens the store tail.
# sum(J_LIST) * P == ROWS. Max J bounded by SBUF: bufs * J*(80+48)B + temps
# must stay under the 192KB/partition Tile cap.
J_LIST = [125, 250, 450, 450, 450, 450, 450, 250, 125, 125]


def build_nc(rows=ROWS, j_list=None, bufs=3):
    """Build the per-core Bass program for a [rows, 20] -> [rows, 12] decode."""
    if j_list is None:
        j_list = J_LIST
    assert sum(j_list) * P == rows, (sum(j_list) * P, rows)
    mult = mybir.AluOpType.mult
    add = mybir.AluOpType.add

    # Bacc (not plain Bass): its compile pipeline runs generate_event_semaphores,
    # which splits multi-wait instructions to the TRN2 1-wait-per-inst limit.
    nc = bacc.Bacc()
    x = nc.dram_tensor("y_pred", [rows, C_IN], F32, kind="ExternalInput")
    y = nc.dram_tensor("out", [rows, C_OUT], F32, kind="ExternalOutput")

    with TileContext(nc) as tc:
        with (
            tc.tile_pool(name="io", bufs=bufs) as io,
            tc.tile_pool(name="tmp", bufs=2) as tp,
        ):
            r0 = 0
            for j in j_list:
                tile_rows = P * j
                xin = x[r0 : r0 + tile_rows, :].rearrange("(p j) c -> p (j c)", p=P)
                xt = io.tile([P, j * C_IN], F32, tag="in")
                nc.sync.dma_start(out=xt[:], in_=xin)
                xv = xt[:].rearrange("p (j c) -> p j c", c=C_IN)

                ot = io.tile([P, j * C_OUT], F32, tag="out")
                ov = ot[:].rearrange("p (j c) -> p j c", c=C_OUT)

                # aw = var_w * 512 * w ; ah = var_h * 512 * h
                aw = tp.tile([P, j], F32, tag="aw")
                ah = tp.tile([P, j], F32, tag="ah")
                nc.vector.scalar_tensor_tensor(
                    out=aw[:], in0=xv[:, :, 16], scalar=SCALE, in1=xv[:, :, 14],
                    op0=mult, op1=mult,
                )
                nc.vector.scalar_tensor_tensor(
                    out=ah[:], in0=xv[:, :, 17], scalar=SCALE, in1=xv[:, :, 15],
                    op0=mult, op1=mult,
                )

                aw_b = aw[:].unsqueeze(2).broadcast_to((P, j, 5))
                ah_b = ah[:].unsqueeze(2).broadcast_to((P, j, 5))
                cx_b = xv[:, :, 12:13].broadcast_to((P, j, 5))
                cy_b = xv[:, :, 13:14].broadcast_to((P, j, 5))

                ox = ov[:, :, 2:12:2]
                oy = ov[:, :, 3:12:2]
                # ox = x_off * aw ; ox = cx*512 + ox  (fused via scalar_tensor_tensor)
                nc.vector.tensor_mul(out=ox, in0=xv[:, :, 2:12:2], in1=aw_b)
                nc.vector.scalar_tensor_tensor(
                    out=ox, in0=cx_b, scalar=SCALE, in1=ox, op0=mult, op1=add,
                )
                nc.vector.tensor_mul(out=oy, in0=xv[:, :, 3:12:2], in1=ah_b)
                nc.vector.scalar_tensor_tensor(
                    out=oy, in0=cy_b, scalar=SCALE, in1=oy, op0=mult, op1=add,
                )

                # class channels pass through, on ScalarE to keep DVE lighter
                # (Bacc's generate_event_semaphores legalizes multi-engine waits)
                nc.scalar.copy(out=ov[:, :, 0:2], in_=xv[:, :, 0:2])

                yout = y[r0 : r0 + tile_rows, :].rearrange("(p j) c -> p (j c)", p=P)
                nc.scalar.dma_start(out=yout, in_=ot[:])
                r0 += tile_rows

    nc.finalize()
    return nc


_NC_CACHE = {}


def _get_nc():
    if "nc" not in _NC_CACHE:
        _NC_CACHE["nc"] = build_nc()
    return _NC_CACHE["nc"]


def kernel(y_pred: np.ndarray) -> np.ndarray:
    from concourse.bass_utils import run_bass_kernel_spmd

    y_pred = np.asarray(y_pred, dtype=np.float32)
    assert y_pred.shape == (B, N, C_IN), y_pred.shape

    nc = _get_nc()
    shards = y_pred.reshape(N_CORES, ROWS, C_IN)
    in_maps = [{"y_pred": shards[c]} for c in range(N_CORES)]
    res = run_bass_kernel_spmd(nc, in_maps, list(range(N_CORES)))
    out = np.stack([res.results[c]["out"] for c in range(N_CORES)])
    return out.reshape(B, N, C_OUT)

